# revision 18
# baseline (speedup 1.0000x reference)
"""MoE-LoRA linear kernel for TRN2, data-parallel over tokens across 8 cores.

Per-core computation (Tc tokens, D=1024, E=10, R=4, TOP_K=2):
  base = x @ W^T + b ; logits = x @ gateW^T + gb ; top2 softmax -> dense w[t,e]
  h = (x @ lora_down^T) * w  (rank-expanded) ; out = base + 0.25 * h @ lora_up^T

v3 design:
- x is transposed AND cast to bf16 on the HOST ([group, 128d, kb, tok]); no
  on-chip transposes, half the input DMA bytes, and bf16 enables the PE's
  fast-weight-load path. W/gate/lora params are bf16 too (gate split hi/lo
  so logits keep ~16 bits of gate precision). Numpy sim on the real data:
  36/16384 tokens misroute, rel err 5.8e-3 (budget 2e-2).
- Down-projection + gate logits merge into ONE 106-column matmul per
  k-block (stationary columns are free; only the 512-token stream costs).
- Output is staged [128tok, 2tt*1024] and stored to a per-partition-
  contiguous DRAM layout (8KB descriptors instead of 4KB), unpacked to
  [Tc, D] on the host. Outputs ride the scalar HWDGE ring; x groups 1-3
  ride the gpsimd SWDGE ring so the sync ring delivers W + group 0 first.
- Identity warmup matmuls flip the PE HAM clock gate to 2.4 GHz while the
  first DMAs land.
"""

import contextlib
import ctypes
import sys
import types

import numpy as np

SO_PATH = "/opt/axon/libaxon_pjrt.so"

D = 1024
E = 10
R = 4
ER = E * R          # 40
GHI0 = 64           # gate-hi lhsT columns (32-aligned partition start for PSUM reads)
GLO0 = 96           # gate-lo lhsT columns
GCOLS = GLO0 + E    # 106 = down(0:40) + pad + gate_hi(64:74) + pad + gate_lo(96:106)
TT_PER_GROUP = 4    # 128-token tiles per 512-token group
TG = 128 * TT_PER_GROUP  # 512 tokens per group
N_WARM = 20         # HAM warmup matmuls (fp32 N=128 ~500ns each)


def install_ntff_hook():
    """run_bass_kernel_spmd(trace=True) needs antenv.axon_hooks; synthesize it."""
    if "antenv.axon_hooks" in sys.modules:
        return
    def _ntff_profile_via_ctypes(so_path):
        lib = ctypes.CDLL(so_path)
        if not hasattr(lib, "axon_start_nrt_profile"):
            return None
        lib.axon_start_nrt_profile.argtypes = [ctypes.POINTER(ctypes.c_int64), ctypes.c_size_t]
        lib.axon_start_nrt_profile.restype = ctypes.c_int64
        lib.axon_stop_nrt_profile.argtypes = [ctypes.c_char_p]
        lib.axon_stop_nrt_profile.restype = ctypes.c_int64

        @contextlib.contextmanager
        def _hook(output_dir, device_ids):
            import jax
            jax.devices()
            if device_ids:
                ids = (ctypes.c_int64 * len(device_ids))(*device_ids)
                rc = lib.axon_start_nrt_profile(ids, len(device_ids))
            else:
                rc = lib.axon_start_nrt_profile(None, 0)
            if rc != 0:
                raise RuntimeError(f"axon_start_nrt_profile rc={rc}")
            try:
                yield
            finally:
                n = lib.axon_stop_nrt_profile(str(output_dir).encode())
                if n < 0:
                    raise RuntimeError(f"axon_stop_nrt_profile rc={n}")
        return _hook

    mod = types.ModuleType("antenv.axon_hooks")
    mod.get_axon_ntff_profile_hook = lambda: _ntff_profile_via_ctypes(SO_PATH)
    sys.modules["antenv.axon_hooks"] = mod


def build_kernel(Tc, n_cores=8):
    import concourse.bass as bass  # noqa: F401
    import concourse.mybir as mybir
    import concourse.tile as tile
    from concourse import bacc
    from concourse.bass import ds, ts
    from concourse.masks import make_identity

    f32 = mybir.dt.float32
    bf16 = mybir.dt.bfloat16
    NG = Tc // TG  # groups of 512 tokens
    assert Tc % TG == 0

    nc = bacc.Bacc("TRN2", target_bir_lowering=False, debug=False, num_devices=n_cores)

    # host-pretransposed bf16 x: xt[gi, p, kb, t] = x[gi*TG + t, kb*128 + p]
    xt_in = nc.declare_dram_parameter("xt", [NG, 128, 8, TG], bf16, isOutput=False)
    wt_in = nc.declare_dram_parameter("wt", [128, 8, D], bf16, isOutput=False)
    g_in = nc.declare_dram_parameter("g", [128, 8, GCOLS], bf16, isOutput=False)
    u_in = nc.declare_dram_parameter("u", [ER + 1, D], bf16, isOutput=False)
    gb_in = nc.declare_dram_parameter("gb", [E, 1], f32, isOutput=False)
    # token-partition-major bf16 output (host casts back to fp32):
    # out_dev[gi, p, tt*D + o] = out[gi*TG + tt*128 + p, o]
    out_dram = nc.declare_dram_parameter(
        "out", [NG, 128, TT_PER_GROUP * D], bf16, isOutput=True)

    with tile.TileContext(nc) as tc:
        with contextlib.ExitStack() as ctx:
            singles = ctx.enter_context(tc.tile_pool(name="singles", bufs=1))
            smallp = ctx.enter_context(tc.tile_pool(name="smallp", bufs=2))
            h1p = ctx.enter_context(tc.tile_pool(name="h1p", bufs=2))
            outp = ctx.enter_context(tc.tile_pool(name="outp", bufs=3))
            po = ctx.enter_context(tc.tile_pool(name="po", bufs=5, space="PSUM"))
            pgp = ctx.enter_context(tc.tile_pool(name="pgp", bufs=1, space="PSUM"))
            pslp = ctx.enter_context(tc.tile_pool(name="pslp", bufs=1, space="PSUM"))
            pswp = ctx.enter_context(tc.tile_pool(name="pswp", bufs=1, space="PSUM"))

            # ---- constants / inputs ----
            g_sb = singles.tile([128, 8, GCOLS], bf16)
            u_sb = singles.tile([ER + 1, D], bf16)
            gb_sb = singles.tile([E, 1], f32)
            ident = singles.tile([128, 128], f32)
            ones = singles.tile([ER + 1, 1], f32)
            make_identity(nc, ident)
            nc.vector.memset(ones[:], 1.0)

            # sync ring: x group 0, then W (needed in token-tile order);
            # gpsimd SWDGE ring: x groups 1..3 (needed later)
            xt_sb = []
            for gi in range(NG):
                x_t = singles.tile([128, 8, TG], bf16, tag=f"xt{gi}")
                xt_sb.append(x_t)
            # ALL traffic on the sync ring, FIFO = priority order (the scalar
            # ring drip-feeds whenever the sync ring is loaded; a 40-byte gb
            # DMA once completed at 29us there and stalled the gating chain)
            nc.sync.dma_start(out=xt_sb[0][:], in_=xt_in[0])
            nc.sync.dma_start(out=g_sb[:], in_=g_in[:])
            nc.sync.dma_start(out=u_sb[:], in_=u_in[:])
            nc.sync.dma_start(out=gb_sb[:], in_=gb_in[:])
            NWC = 4  # W chunks (2 k-blocks each, 0.5 MB)
            wt_sb = []
            for c in range(NWC):
                w_t = singles.tile([128, 8 // NWC, D], bf16, tag=f"wt{c}")
                wt_sb.append(w_t)
                nc.sync.dma_start(out=w_t[:], in_=wt_in[:, ds(c * (8 // NWC), 8 // NWC), :])
            # later x groups follow W on the same FIFO ring: W is the
            # critical input, groups 1-3 aren't needed until 30/48/66us
            for gi in range(1, NG):
                nc.sync.dma_start(out=xt_sb[gi][:], in_=xt_in[gi])

            def wt_slice(kb, ch):
                c, kbi = divmod(kb, 8 // NWC)
                return wt_sb[c][:, kbi, ds(ch * 512, 512)]

            # ---- HAM warmup: fp32 identity matmuls (~500ns each) bridge the
            # PE from t~3us until the first x group lands (~11us), flipping
            # the HAM clock gate to 2.4GHz before real work starts ----
            warm_ps = pslp.tile([128, 128], f32, tag="psl")
            for _ in range(N_WARM):
                nc.tensor.matmul(warm_ps[:], ident, ident, start=True, stop=True)

            # ---- per-group phase A+B, software-pipelined into phase C ----
            def dg_matmul(gi):
                """down+gate merged matmul: pg rows 0:40 = h, 64:74/96:106 = logits hi/lo."""
                pg = pgp.tile([GCOLS, TG], f32, tag="pg")
                for kb in range(8):
                    nc.tensor.matmul(
                        pg[:], g_sb[:, kb, :], xt_sb[gi][:, kb, :],
                        start=(kb == 0), stop=(kb == 7),
                    )
                return pg

            def gate_front(gi, pg):
                """logits -> token-major psl transposes (PE) + hr evac."""
                hr = smallp.tile([ER, TG], f32, tag="hr")
                nc.scalar.copy(hr[:], pg[0:ER, :])
                # logits_hi + gate bias during PSUM evacuation (ACT), then +lo
                lt = smallp.tile([E, TG], f32, tag="lt")
                nc.scalar.activation(
                    lt[:], pg[GHI0:GHI0 + E, :],
                    mybir.ActivationFunctionType.Identity, bias=gb_sb[:],
                )
                lt2 = smallp.tile([E, TG], f32, tag="lt2")
                nc.vector.tensor_tensor(
                    lt2[:], lt[:], pg[GLO0:GLO0 + E, :], mybir.AluOpType.add,
                )
                psl = pslp.tile([128, TT_PER_GROUP, E], f32, tag="psl")
                for tt in range(TT_PER_GROUP):
                    nc.tensor.transpose(
                        psl[:, tt, :], lt2[:, ts(tt, 128)], ident[0:E, 0:E],
                    )
                return hr, psl

            def gate_back(gi, hr, psl):
                """top-2 softmax on tokens, rank-expand, transpose back, weight h."""
                L = smallp.tile([128, TT_PER_GROUP, E], f32, tag="L")
                nc.vector.tensor_copy(L[:], psl[:])
                m1 = smallp.tile([128, TT_PER_GROUP], f32, tag="m1")
                nc.vector.reduce_max(m1[:], L[:], axis=mybir.AxisListType.X)
                Lm = smallp.tile([128, TT_PER_GROUP, E], f32, tag="Lm")
                nc.vector.tensor_tensor(
                    Lm[:], L[:], m1[:, :, None].to_broadcast(L.shape),
                    mybir.AluOpType.subtract,
                )
                mmax = smallp.tile([128, TT_PER_GROUP, E], f32, tag="mmax")
                nc.vector.tensor_scalar(
                    mmax[:], Lm[:], 0.0, None, op0=mybir.AluOpType.is_equal,
                )
                nc.vector.tensor_scalar_mul(mmax[:], mmax[:], -1e30)
                nc.vector.tensor_tensor(mmax[:], Lm[:], mmax[:], mybir.AluOpType.add)
                m2 = smallp.tile([128, TT_PER_GROUP], f32, tag="m2")
                nc.vector.reduce_max(m2[:], mmax[:], axis=mybir.AxisListType.X)
                mask2 = smallp.tile([128, TT_PER_GROUP, E], f32, tag="mask2")
                nc.vector.tensor_tensor(
                    mask2[:], Lm[:], m2[:, :, None].to_broadcast(Lm.shape),
                    mybir.AluOpType.is_ge,
                )
                ex = smallp.tile([128, TT_PER_GROUP, E], f32, tag="ex")
                nc.scalar.activation(ex[:], Lm[:], mybir.ActivationFunctionType.Exp)
                nc.vector.tensor_tensor(ex[:], ex[:], mask2[:], mybir.AluOpType.mult)
                zsum = smallp.tile([128, TT_PER_GROUP], f32, tag="zsum")
                nc.vector.reduce_sum(zsum[:], ex[:], axis=mybir.AxisListType.X)
                nc.vector.reciprocal(zsum[:], zsum[:])
                wfull = smallp.tile([128, TT_PER_GROUP, E], f32, tag="wfull")
                nc.vector.tensor_tensor(
                    wfull[:], ex[:], zsum[:, :, None].to_broadcast(ex.shape),
                    mybir.AluOpType.mult,
                )
                w40 = smallp.tile([128, TT_PER_GROUP, ER], f32, tag="w40")
                nc.vector.tensor_copy(
                    w40[:],
                    wfull[:, :, :, None].to_broadcast([128, TT_PER_GROUP, E, R]),
                )
                psw = pswp.tile([ER, TG], f32, tag="psw")
                for tt in range(TT_PER_GROUP):
                    nc.tensor.transpose(
                        psw[:, ts(tt, 128)], w40[:, tt, :], ident,
                    )
                # h1 rows 0:40 = hr * w (bf16); row 40 = 1.0 (bias row of u)
                h1 = h1p.tile([ER + 1, TG], bf16, tag="h1")
                nc.vector.tensor_copy(h1[:], ones.to_broadcast([ER + 1, TG]))
                nc.vector.tensor_tensor(
                    h1[0:ER, :], hr[:], psw[:], mybir.AluOpType.mult,
                )
                return h1

            def token_tile(gi, tt, h1, o_sb, tj):
                """one 128-token tile: base matmuls + LoRA-up; evac split
                across ACT and DVE so neither queue's stalls gate PSUM
                bank recycling."""
                pout0 = po.tile([128, 512], f32, tag="po")
                pout1 = po.tile([128, 512], f32, tag="po")
                pouts = [pout0, pout1]
                for kb in range(8):
                    for ch in range(2):
                        nc.tensor.matmul(
                            pouts[ch][:], xt_sb[gi][:, kb, ts(tt, 128)],
                            wt_slice(kb, ch),
                            start=(kb == 0), stop=False,
                        )
                for ch in range(2):
                    nc.tensor.matmul(
                        pouts[ch][:], h1[:, ts(tt, 128)], u_sb[:, ds(ch * 512, 512)],
                        start=False, stop=True,
                    )
                nc.scalar.copy(o_sb[:, tj, ds(0, 512)], pouts[0][:])
                nc.vector.tensor_copy(o_sb[:, tj, ds(512, 512)], pouts[1][:])

            def pair_store(gi, ttp, o_sb):
                nc.sync.dma_start(
                    out=out_dram[gi, :, ds(ttp * 2 * D, 2 * D)], in_=o_sb[:],
                )

            # prologue: group 0 gating (latency hides under the W-chunk loads)
            pg = dg_matmul(0)
            hr, psl = gate_front(0, pg)
            h1 = gate_back(0, hr, psl)
            for gi in range(NG):
                o_sb = outp.tile([128, 2, D], bf16, tag="o_sb")
                token_tile(gi, 0, h1, o_sb, 0)
                token_tile(gi, 1, h1, o_sb, 1)
                pair_store(gi, 0, o_sb)
                if gi + 1 < NG:
                    pg_n = dg_matmul(gi + 1)
                    hr_n, psl_n = gate_front(gi + 1, pg_n)
                    h1_n = gate_back(gi + 1, hr_n, psl_n)
                o_sb = outp.tile([128, 2, D], bf16, tag="o_sb")
                token_tile(gi, 2, h1, o_sb, 0)
                token_tile(gi, 3, h1, o_sb, 1)
                pair_store(gi, 1, o_sb)
                if gi + 1 < NG:
                    h1 = h1_n

    nc.compile()
    return nc


def pack_weights(W_base, b_base, gate_W, gate_b, lora_down, lora_up):
    """Host-side packing of the replicated weights into device layouts (bf16)."""
    import ml_dtypes
    bf = ml_dtypes.bfloat16
    W_base = np.asarray(W_base, np.float32)
    b_base = np.asarray(b_base, np.float32)
    gate_W = np.asarray(gate_W, np.float32)
    gate_b = np.asarray(gate_b, np.float32)
    lora_down = np.asarray(lora_down, np.float32)
    lora_up = np.asarray(lora_up, np.float32)

    # wt[p, kb, o] = W^T[d, o] = W_base[o, d], d = kb*128+p
    wt = np.ascontiguousarray(
        np.ascontiguousarray(W_base.T).reshape(8, 128, D).transpose(1, 0, 2)
    ).astype(bf)
    # merged lhsT: cols 0..39 lora_down^T, 64..73 gate_hi, 96..105 gate_lo.
    # gate_W is split hi/lo in bf16 so the two column sets sum to ~16-bit
    # gate precision (x itself is bf16).
    gwT = np.ascontiguousarray(gate_W.T)                  # [D, E]
    gw_hi = gwT.astype(bf).astype(np.float32)
    gw_lo = (gwT - gw_hi).astype(bf).astype(np.float32)
    G = np.zeros((D, GCOLS), np.float32)
    G[:, 0:ER] = lora_down.reshape(ER, D).T
    G[:, GHI0:GHI0 + E] = gw_hi
    G[:, GLO0:GLO0 + E] = gw_lo
    g = np.ascontiguousarray(G.reshape(8, 128, GCOLS).transpose(1, 0, 2)).astype(bf)
    # u rows 0..39: lora_up[e, o, r]*0.25 -> [er, o]; row 40: b_base
    U = lora_up.transpose(0, 2, 1).reshape(ER, D) * (1.0 / R)
    u = np.ascontiguousarray(np.concatenate([U, b_base[None, :]], axis=0)).astype(bf)
    gb = np.ascontiguousarray(gate_b[:, None])
    return {"wt": wt, "g": g, "u": u, "gb": gb}


def pack_x(xc, Tc):
    """Per-core x [Tc, D] -> bf16 [NG, 128, 8, TG]: xt[gi,p,kb,t]=x[gi*TG+t, kb*128+p]."""
    import ml_dtypes
    NG = Tc // TG
    return np.ascontiguousarray(
        xc.reshape(NG, TG, 8, 128).transpose(0, 3, 2, 1)
    ).astype(ml_dtypes.bfloat16)


def unpack_out(o_dev, Tc):
    """Device layout bf16 [NG, 128, TT*D] -> fp32 [Tc, D]."""
    NG = Tc // TG
    return np.ascontiguousarray(
        o_dev.reshape(NG, 128, TT_PER_GROUP, D).transpose(0, 2, 1, 3)
    ).reshape(Tc, D).astype(np.float32)


def run(nc, inputs, Tc, n_cores=8, trace=False):
    """Shard x over cores, run SPMD, gather output."""
    from concourse.bass_utils import run_bass_kernel_spmd

    x = np.asarray(inputs["x"], np.float32)
    B, S, _ = x.shape
    xf = x.reshape(B * S, D)
    assert B * S == Tc * n_cores
    packed = pack_weights(
        inputs["W_base"], inputs["b_base"], inputs["gate_W"],
        inputs["gate_b"], inputs["lora_down"], inputs["lora_up"],
    )
    in_maps = [
        {"xt": pack_x(xf[c * Tc:(c + 1) * Tc], Tc), **packed}
        for c in range(n_cores)
    ]
    kwargs = {}
    if trace:
        install_ntff_hook()
        kwargs = {"trace": True}
    res = run_bass_kernel_spmd(nc, in_maps, core_ids=list(range(n_cores)), **kwargs)
    out = np.concatenate(
        [unpack_out(res.results[c]["out"], Tc) for c in range(n_cores)], axis=0)
    return out.reshape(B, S, D), res


_NC_CACHE = {}


def kernel(**inputs):
    """Full-input MoE-LoRA forward on 8 TRN2 NeuronCores (token-parallel).

    Takes the unsharded inputs from setup_inputs(), returns [B, S, D] fp32.
    """
    x = np.asarray(inputs["x"], np.float32)
    B, S, _ = x.shape
    n_cores = 8
    total = B * S
    assert total % n_cores == 0
    Tc = total // n_cores
    key = (Tc, n_cores)
    if key not in _NC_CACHE:
        _NC_CACHE[key] = build_kernel(Tc, n_cores=n_cores)
    nc = _NC_CACHE[key]
    last_err = None
    for _ in range(3):  # transient device wedges recover on retry
        try:
            out, _res = run(nc, inputs, Tc, n_cores=n_cores)
            return out
        except Exception as e:  # noqa: BLE001
            last_err = e
            import time as _time
            _time.sleep(5)
    raise last_err


# revision 19
# speedup vs baseline: 1.1563x; 1.1563x over previous
"""MoE-LoRA linear kernel for TRN2, data-parallel over tokens across 8 cores.

Per-core computation (Tc tokens, D=1024, E=10, R=4, TOP_K=2):
  base = x @ W^T + b ; logits = x @ gateW^T + gb ; top2 softmax -> dense w[t,e]
  h = (x @ lora_down^T) * w  (rank-expanded) ; out = base + 0.25 * h @ lora_up^T

v3 design:
- x is transposed AND cast to bf16 on the HOST ([group, 128d, kb, tok]); no
  on-chip transposes, half the input DMA bytes, and bf16 enables the PE's
  fast-weight-load path. W/gate/lora params are bf16 too (gate split hi/lo
  so logits keep ~16 bits of gate precision). Numpy sim on the real data:
  36/16384 tokens misroute, rel err 5.8e-3 (budget 2e-2).
- Down-projection + gate logits merge into ONE 106-column matmul per
  k-block (stationary columns are free; only the 512-token stream costs).
- Output is staged [128tok, 2tt*1024] and stored to a per-partition-
  contiguous DRAM layout (8KB descriptors instead of 4KB), unpacked to
  [Tc, D] on the host. Outputs ride the scalar HWDGE ring; x groups 1-3
  ride the gpsimd SWDGE ring so the sync ring delivers W + group 0 first.
- Identity warmup matmuls flip the PE HAM clock gate to 2.4 GHz while the
  first DMAs land.
"""

import contextlib
import ctypes
import sys
import types

import numpy as np

SO_PATH = "/opt/axon/libaxon_pjrt.so"

D = 1024
E = 10
R = 4
ER = E * R          # 40
GHI0 = 64           # gate-hi lhsT columns (32-aligned partition start for PSUM reads)
GLO0 = 96           # gate-lo lhsT columns
GCOLS = GLO0 + E    # 106 = down(0:40) + pad + gate_hi(64:74) + pad + gate_lo(96:106)
TT_PER_GROUP = 4    # 128-token tiles per 512-token group
TG = 128 * TT_PER_GROUP  # 512 tokens per group
N_WARM = 20         # HAM warmup matmuls (fp32 N=128 ~500ns each)


def install_ntff_hook():
    """run_bass_kernel_spmd(trace=True) needs antenv.axon_hooks; synthesize it."""
    if "antenv.axon_hooks" in sys.modules:
        return
    def _ntff_profile_via_ctypes(so_path):
        lib = ctypes.CDLL(so_path)
        if not hasattr(lib, "axon_start_nrt_profile"):
            return None
        lib.axon_start_nrt_profile.argtypes = [ctypes.POINTER(ctypes.c_int64), ctypes.c_size_t]
        lib.axon_start_nrt_profile.restype = ctypes.c_int64
        lib.axon_stop_nrt_profile.argtypes = [ctypes.c_char_p]
        lib.axon_stop_nrt_profile.restype = ctypes.c_int64

        @contextlib.contextmanager
        def _hook(output_dir, device_ids):
            import jax
            jax.devices()
            if device_ids:
                ids = (ctypes.c_int64 * len(device_ids))(*device_ids)
                rc = lib.axon_start_nrt_profile(ids, len(device_ids))
            else:
                rc = lib.axon_start_nrt_profile(None, 0)
            if rc != 0:
                raise RuntimeError(f"axon_start_nrt_profile rc={rc}")
            try:
                yield
            finally:
                n = lib.axon_stop_nrt_profile(str(output_dir).encode())
                if n < 0:
                    raise RuntimeError(f"axon_stop_nrt_profile rc={n}")
        return _hook

    mod = types.ModuleType("antenv.axon_hooks")
    mod.get_axon_ntff_profile_hook = lambda: _ntff_profile_via_ctypes(SO_PATH)
    sys.modules["antenv.axon_hooks"] = mod


def build_kernel(Tc, n_cores=8):
    import concourse.bass as bass  # noqa: F401
    import concourse.mybir as mybir
    import concourse.tile as tile
    from concourse import bacc
    from concourse.bass import ds, ts
    from concourse.masks import make_identity

    f32 = mybir.dt.float32
    bf16 = mybir.dt.bfloat16
    NG = Tc // TG  # groups of 512 tokens
    assert Tc % TG == 0

    nc = bacc.Bacc("TRN2", target_bir_lowering=False, debug=False, num_devices=n_cores)

    # host-pretransposed bf16 x: xt[gi, p, kb, t] = x[gi*TG + t, kb*128 + p]
    xt_in = nc.declare_dram_parameter("xt", [NG, 128, 8, TG], bf16, isOutput=False)
    wt_in = nc.declare_dram_parameter("wt", [128, 8, D], bf16, isOutput=False)
    g_in = nc.declare_dram_parameter("g", [128, 8, GCOLS], bf16, isOutput=False)
    u_in = nc.declare_dram_parameter("u", [ER + 1, D], bf16, isOutput=False)
    gb_in = nc.declare_dram_parameter("gb", [E, 1], f32, isOutput=False)
    # token-partition-major bf16 output (host casts back to fp32):
    # out_dev[gi, p, tt*D + o] = out[gi*TG + tt*128 + p, o]
    out_dram = nc.declare_dram_parameter(
        "out", [NG, 128, TT_PER_GROUP * D], bf16, isOutput=True)

    with tile.TileContext(nc) as tc:
        with contextlib.ExitStack() as ctx:
            singles = ctx.enter_context(tc.tile_pool(name="singles", bufs=1))
            smallp = ctx.enter_context(tc.tile_pool(name="smallp", bufs=2))
            h1p = ctx.enter_context(tc.tile_pool(name="h1p", bufs=2))
            outp = ctx.enter_context(tc.tile_pool(name="outp", bufs=3))
            po = ctx.enter_context(tc.tile_pool(name="po", bufs=5, space="PSUM"))
            pgp = ctx.enter_context(tc.tile_pool(name="pgp", bufs=1, space="PSUM"))
            pslp = ctx.enter_context(tc.tile_pool(name="pslp", bufs=1, space="PSUM"))
            pswp = ctx.enter_context(tc.tile_pool(name="pswp", bufs=1, space="PSUM"))

            # ---- constants / inputs ----
            g_sb = singles.tile([128, 8, GCOLS], bf16)
            u_sb = singles.tile([ER + 1, D], bf16)
            gb_sb = singles.tile([E, 1], f32)
            ident = singles.tile([128, 128], f32)
            ones = singles.tile([ER + 1, 1], f32)
            make_identity(nc, ident)
            nc.vector.memset(ones[:], 1.0)

            # sync ring: x group 0, then W (needed in token-tile order);
            # gpsimd SWDGE ring: x groups 1..3 (needed later)
            xt_sb = []
            for gi in range(NG):
                x_t = singles.tile([128, 8, TG], bf16, tag=f"xt{gi}")
                xt_sb.append(x_t)
            # ALL traffic on the sync ring, FIFO = priority order (the scalar
            # ring drip-feeds whenever the sync ring is loaded; a 40-byte gb
            # DMA once completed at 29us there and stalled the gating chain)
            nc.sync.dma_start(out=xt_sb[0][:], in_=xt_in[0])
            nc.sync.dma_start(out=g_sb[:], in_=g_in[:])
            nc.sync.dma_start(out=u_sb[:], in_=u_in[:])
            nc.sync.dma_start(out=gb_sb[:], in_=gb_in[:])
            NWC = 4  # W chunks (2 k-blocks each, 0.5 MB)
            wt_sb = []
            for c in range(NWC):
                w_t = singles.tile([128, 8 // NWC, D], bf16, tag=f"wt{c}")
                wt_sb.append(w_t)
                nc.sync.dma_start(out=w_t[:], in_=wt_in[:, ds(c * (8 // NWC), 8 // NWC), :])
            # later x groups follow W on the same FIFO ring: W is the
            # critical input, groups 1-3 aren't needed until 30/48/66us
            for gi in range(1, NG):
                nc.sync.dma_start(out=xt_sb[gi][:], in_=xt_in[gi])

            def wt_slice(kb, ch):
                c, kbi = divmod(kb, 8 // NWC)
                return wt_sb[c][:, kbi, ds(ch * 512, 512)]

            # ---- HAM warmup: fp32 identity matmuls (~500ns each) bridge the
            # PE from t~3us until the first x group lands (~11us), flipping
            # the HAM clock gate to 2.4GHz before real work starts ----
            warm_ps = pslp.tile([128, 128], f32, tag="psl")
            for _ in range(N_WARM):
                nc.tensor.matmul(warm_ps[:], ident, ident, start=True, stop=True)

            # ---- per-group phase A+B, software-pipelined into phase C ----
            def dg_matmul(gi):
                """down+gate merged matmul: pg rows 0:40 = h, 64:74/96:106 = logits hi/lo."""
                pg = pgp.tile([GCOLS, TG], f32, tag="pg")
                for kb in range(8):
                    nc.tensor.matmul(
                        pg[:], g_sb[:, kb, :], xt_sb[gi][:, kb, :],
                        start=(kb == 0), stop=(kb == 7),
                    )
                return pg

            def gate_front(gi, pg):
                """logits -> token-major psl transposes (PE) + hr evac."""
                hr = smallp.tile([ER, TG], f32, tag="hr")
                nc.scalar.copy(hr[:], pg[0:ER, :])
                # logits_hi + gate bias during PSUM evacuation (ACT), then +lo
                lt = smallp.tile([E, TG], f32, tag="lt")
                nc.scalar.activation(
                    lt[:], pg[GHI0:GHI0 + E, :],
                    mybir.ActivationFunctionType.Identity, bias=gb_sb[:],
                )
                lt2 = smallp.tile([E, TG], f32, tag="lt2")
                nc.vector.tensor_tensor(
                    lt2[:], lt[:], pg[GLO0:GLO0 + E, :], mybir.AluOpType.add,
                )
                psl = pslp.tile([128, TT_PER_GROUP, E], f32, tag="psl")
                for tt in range(TT_PER_GROUP):
                    nc.tensor.transpose(
                        psl[:, tt, :], lt2[:, ts(tt, 128)], ident[0:E, 0:E],
                    )
                return hr, psl

            def gate_back(gi, hr, psl):
                """top-2 softmax on tokens, rank-expand, transpose back, weight h."""
                L = smallp.tile([128, TT_PER_GROUP, E], f32, tag="L")
                nc.vector.tensor_copy(L[:], psl[:])
                m1 = smallp.tile([128, TT_PER_GROUP], f32, tag="m1")
                nc.vector.reduce_max(m1[:], L[:], axis=mybir.AxisListType.X)
                Lm = smallp.tile([128, TT_PER_GROUP, E], f32, tag="Lm")
                nc.vector.tensor_tensor(
                    Lm[:], L[:], m1[:, :, None].to_broadcast(L.shape),
                    mybir.AluOpType.subtract,
                )
                mmax = smallp.tile([128, TT_PER_GROUP, E], f32, tag="mmax")
                nc.vector.tensor_scalar(
                    mmax[:], Lm[:], 0.0, None, op0=mybir.AluOpType.is_equal,
                )
                nc.vector.tensor_scalar_mul(mmax[:], mmax[:], -1e30)
                nc.vector.tensor_tensor(mmax[:], Lm[:], mmax[:], mybir.AluOpType.add)
                m2 = smallp.tile([128, TT_PER_GROUP], f32, tag="m2")
                nc.vector.reduce_max(m2[:], mmax[:], axis=mybir.AxisListType.X)
                mask2 = smallp.tile([128, TT_PER_GROUP, E], f32, tag="mask2")
                nc.vector.tensor_tensor(
                    mask2[:], Lm[:], m2[:, :, None].to_broadcast(Lm.shape),
                    mybir.AluOpType.is_ge,
                )
                ex = smallp.tile([128, TT_PER_GROUP, E], f32, tag="ex")
                nc.scalar.activation(ex[:], Lm[:], mybir.ActivationFunctionType.Exp)
                nc.vector.tensor_tensor(ex[:], ex[:], mask2[:], mybir.AluOpType.mult)
                zsum = smallp.tile([128, TT_PER_GROUP], f32, tag="zsum")
                nc.vector.reduce_sum(zsum[:], ex[:], axis=mybir.AxisListType.X)
                nc.vector.reciprocal(zsum[:], zsum[:])
                wfull = smallp.tile([128, TT_PER_GROUP, E], f32, tag="wfull")
                nc.vector.tensor_tensor(
                    wfull[:], ex[:], zsum[:, :, None].to_broadcast(ex.shape),
                    mybir.AluOpType.mult,
                )
                w40 = smallp.tile([128, TT_PER_GROUP, ER], f32, tag="w40")
                nc.vector.tensor_copy(
                    w40[:],
                    wfull[:, :, :, None].to_broadcast([128, TT_PER_GROUP, E, R]),
                )
                psw = pswp.tile([ER, TG], f32, tag="psw")
                for tt in range(TT_PER_GROUP):
                    nc.tensor.transpose(
                        psw[:, ts(tt, 128)], w40[:, tt, :], ident,
                    )
                # h1 rows 0:40 = hr * w (bf16); row 40 = 1.0 (bias row of u)
                h1 = h1p.tile([ER + 1, TG], bf16, tag="h1")
                nc.vector.tensor_copy(h1[:], ones.to_broadcast([ER + 1, TG]))
                nc.vector.tensor_tensor(
                    h1[0:ER, :], hr[:], psw[:], mybir.AluOpType.mult,
                )
                return h1

            def token_tile(gi, tt, h1, o_sb, tj):
                """one 128-token tile: base matmuls + LoRA-up; evac split
                across ACT and DVE so neither queue's stalls gate PSUM
                bank recycling."""
                pout0 = po.tile([128, 512], f32, tag="po")
                pout1 = po.tile([128, 512], f32, tag="po")
                pouts = [pout0, pout1]
                for kb in range(8):
                    for ch in range(2):
                        nc.tensor.matmul(
                            pouts[ch][:], xt_sb[gi][:, kb, ts(tt, 128)],
                            wt_slice(kb, ch),
                            start=(kb == 0), stop=False,
                        )
                for ch in range(2):
                    nc.tensor.matmul(
                        pouts[ch][:], h1[:, ts(tt, 128)], u_sb[:, ds(ch * 512, 512)],
                        start=False, stop=True,
                    )
                nc.scalar.copy(o_sb[:, tj, ds(0, 512)], pouts[0][:])
                nc.vector.tensor_copy(o_sb[:, tj, ds(512, 512)], pouts[1][:])

            def pair_store(gi, ttp, o_sb):
                if gi == NG - 1 and ttp == 1:
                    # final store is the kernel tail: split partition halves
                    # across the sync + gpsimd rings so they transfer in
                    # parallel (~halves the tail latency)
                    nc.sync.dma_start(
                        out=out_dram[gi, 0:64, ds(ttp * 2 * D, 2 * D)],
                        in_=o_sb[0:64, :, :],
                    )
                    nc.gpsimd.dma_start(
                        out=out_dram[gi, 64:128, ds(ttp * 2 * D, 2 * D)],
                        in_=o_sb[64:128, :, :],
                    )
                else:
                    nc.sync.dma_start(
                        out=out_dram[gi, :, ds(ttp * 2 * D, 2 * D)], in_=o_sb[:],
                    )

            # prologue: group 0 gating (latency hides under the W-chunk loads)
            pg = dg_matmul(0)
            hr, psl = gate_front(0, pg)
            h1 = gate_back(0, hr, psl)
            for gi in range(NG):
                o_sb = outp.tile([128, 2, D], bf16, tag="o_sb")
                token_tile(gi, 0, h1, o_sb, 0)
                token_tile(gi, 1, h1, o_sb, 1)
                pair_store(gi, 0, o_sb)
                if gi + 1 < NG:
                    pg_n = dg_matmul(gi + 1)
                    hr_n, psl_n = gate_front(gi + 1, pg_n)
                    h1_n = gate_back(gi + 1, hr_n, psl_n)
                o_sb = outp.tile([128, 2, D], bf16, tag="o_sb")
                token_tile(gi, 2, h1, o_sb, 0)
                token_tile(gi, 3, h1, o_sb, 1)
                pair_store(gi, 1, o_sb)
                if gi + 1 < NG:
                    h1 = h1_n

    nc.compile()
    return nc


def pack_weights(W_base, b_base, gate_W, gate_b, lora_down, lora_up):
    """Host-side packing of the replicated weights into device layouts (bf16)."""
    import ml_dtypes
    bf = ml_dtypes.bfloat16
    W_base = np.asarray(W_base, np.float32)
    b_base = np.asarray(b_base, np.float32)
    gate_W = np.asarray(gate_W, np.float32)
    gate_b = np.asarray(gate_b, np.float32)
    lora_down = np.asarray(lora_down, np.float32)
    lora_up = np.asarray(lora_up, np.float32)

    # wt[p, kb, o] = W^T[d, o] = W_base[o, d], d = kb*128+p
    wt = np.ascontiguousarray(
        np.ascontiguousarray(W_base.T).reshape(8, 128, D).transpose(1, 0, 2)
    ).astype(bf)
    # merged lhsT: cols 0..39 lora_down^T, 64..73 gate_hi, 96..105 gate_lo.
    # gate_W is split hi/lo in bf16 so the two column sets sum to ~16-bit
    # gate precision (x itself is bf16).
    gwT = np.ascontiguousarray(gate_W.T)                  # [D, E]
    gw_hi = gwT.astype(bf).astype(np.float32)
    gw_lo = (gwT - gw_hi).astype(bf).astype(np.float32)
    G = np.zeros((D, GCOLS), np.float32)
    G[:, 0:ER] = lora_down.reshape(ER, D).T
    G[:, GHI0:GHI0 + E] = gw_hi
    G[:, GLO0:GLO0 + E] = gw_lo
    g = np.ascontiguousarray(G.reshape(8, 128, GCOLS).transpose(1, 0, 2)).astype(bf)
    # u rows 0..39: lora_up[e, o, r]*0.25 -> [er, o]; row 40: b_base
    U = lora_up.transpose(0, 2, 1).reshape(ER, D) * (1.0 / R)
    u = np.ascontiguousarray(np.concatenate([U, b_base[None, :]], axis=0)).astype(bf)
    gb = np.ascontiguousarray(gate_b[:, None])
    return {"wt": wt, "g": g, "u": u, "gb": gb}


def pack_x(xc, Tc):
    """Per-core x [Tc, D] -> bf16 [NG, 128, 8, TG]: xt[gi,p,kb,t]=x[gi*TG+t, kb*128+p]."""
    import ml_dtypes
    NG = Tc // TG
    return np.ascontiguousarray(
        xc.reshape(NG, TG, 8, 128).transpose(0, 3, 2, 1)
    ).astype(ml_dtypes.bfloat16)


def unpack_out(o_dev, Tc):
    """Device layout bf16 [NG, 128, TT*D] -> fp32 [Tc, D]."""
    NG = Tc // TG
    return np.ascontiguousarray(
        o_dev.reshape(NG, 128, TT_PER_GROUP, D).transpose(0, 2, 1, 3)
    ).reshape(Tc, D).astype(np.float32)


def run(nc, inputs, Tc, n_cores=8, trace=False):
    """Shard x over cores, run SPMD, gather output."""
    from concourse.bass_utils import run_bass_kernel_spmd

    x = np.asarray(inputs["x"], np.float32)
    B, S, _ = x.shape
    xf = x.reshape(B * S, D)
    assert B * S == Tc * n_cores
    packed = pack_weights(
        inputs["W_base"], inputs["b_base"], inputs["gate_W"],
        inputs["gate_b"], inputs["lora_down"], inputs["lora_up"],
    )
    in_maps = [
        {"xt": pack_x(xf[c * Tc:(c + 1) * Tc], Tc), **packed}
        for c in range(n_cores)
    ]
    kwargs = {}
    if trace:
        install_ntff_hook()
        kwargs = {"trace": True}
    res = run_bass_kernel_spmd(nc, in_maps, core_ids=list(range(n_cores)), **kwargs)
    out = np.concatenate(
        [unpack_out(res.results[c]["out"], Tc) for c in range(n_cores)], axis=0)
    return out.reshape(B, S, D), res


_NC_CACHE = {}


def kernel(**inputs):
    """Full-input MoE-LoRA forward on 8 TRN2 NeuronCores (token-parallel).

    Takes the unsharded inputs from setup_inputs(), returns [B, S, D] fp32.
    """
    x = np.asarray(inputs["x"], np.float32)
    B, S, _ = x.shape
    n_cores = 8
    total = B * S
    assert total % n_cores == 0
    Tc = total // n_cores
    key = (Tc, n_cores)
    if key not in _NC_CACHE:
        _NC_CACHE[key] = build_kernel(Tc, n_cores=n_cores)
    nc = _NC_CACHE[key]
    last_err = None
    for _ in range(3):  # transient device wedges recover on retry
        try:
            out, _res = run(nc, inputs, Tc, n_cores=n_cores)
            return out
        except Exception as e:  # noqa: BLE001
            last_err = e
            import time as _time
            _time.sleep(5)
    raise last_err


# revision 20
# speedup vs baseline: 1.1588x; 1.0021x over previous
"""MoE-LoRA linear kernel for TRN2, data-parallel over tokens across 8 cores.

Per-core computation (Tc tokens, D=1024, E=10, R=4, TOP_K=2):
  base = x @ W^T + b ; logits = x @ gateW^T + gb ; top2 softmax -> dense w[t,e]
  h = (x @ lora_down^T) * w  (rank-expanded) ; out = base + 0.25 * h @ lora_up^T

v3 design:
- x is transposed AND cast to bf16 on the HOST ([group, 128d, kb, tok]); no
  on-chip transposes, half the input DMA bytes, and bf16 enables the PE's
  fast-weight-load path. W/gate/lora params are bf16 too (gate split hi/lo
  so logits keep ~16 bits of gate precision). Numpy sim on the real data:
  36/16384 tokens misroute, rel err 5.8e-3 (budget 2e-2).
- Down-projection + gate logits merge into ONE 106-column matmul per
  k-block (stationary columns are free; only the 512-token stream costs).
- Output is staged [128tok, 2tt*1024] and stored to a per-partition-
  contiguous DRAM layout (8KB descriptors instead of 4KB), unpacked to
  [Tc, D] on the host. Outputs ride the scalar HWDGE ring; x groups 1-3
  ride the gpsimd SWDGE ring so the sync ring delivers W + group 0 first.
- Identity warmup matmuls flip the PE HAM clock gate to 2.4 GHz while the
  first DMAs land.
"""

import contextlib
import ctypes
import sys
import types

import numpy as np

SO_PATH = "/opt/axon/libaxon_pjrt.so"

D = 1024
E = 10
R = 4
ER = E * R          # 40
GHI0 = 64           # gate-hi lhsT columns (32-aligned partition start for PSUM reads)
GLO0 = 96           # gate-lo lhsT columns
GCOLS = GLO0 + E    # 106 = down(0:40) + pad + gate_hi(64:74) + pad + gate_lo(96:106)
TT_PER_GROUP = 4    # 128-token tiles per 512-token group
TG = 128 * TT_PER_GROUP  # 512 tokens per group
N_WARM = 10         # HAM warmup matmuls (fp32 N=128 ~500ns each; start ~7.7us after NEFF setup, bridge to x arrival ~12.5us)


def install_ntff_hook():
    """run_bass_kernel_spmd(trace=True) needs antenv.axon_hooks; synthesize it."""
    if "antenv.axon_hooks" in sys.modules:
        return
    def _ntff_profile_via_ctypes(so_path):
        lib = ctypes.CDLL(so_path)
        if not hasattr(lib, "axon_start_nrt_profile"):
            return None
        lib.axon_start_nrt_profile.argtypes = [ctypes.POINTER(ctypes.c_int64), ctypes.c_size_t]
        lib.axon_start_nrt_profile.restype = ctypes.c_int64
        lib.axon_stop_nrt_profile.argtypes = [ctypes.c_char_p]
        lib.axon_stop_nrt_profile.restype = ctypes.c_int64

        @contextlib.contextmanager
        def _hook(output_dir, device_ids):
            import jax
            jax.devices()
            if device_ids:
                ids = (ctypes.c_int64 * len(device_ids))(*device_ids)
                rc = lib.axon_start_nrt_profile(ids, len(device_ids))
            else:
                rc = lib.axon_start_nrt_profile(None, 0)
            if rc != 0:
                raise RuntimeError(f"axon_start_nrt_profile rc={rc}")
            try:
                yield
            finally:
                n = lib.axon_stop_nrt_profile(str(output_dir).encode())
                if n < 0:
                    raise RuntimeError(f"axon_stop_nrt_profile rc={n}")
        return _hook

    mod = types.ModuleType("antenv.axon_hooks")
    mod.get_axon_ntff_profile_hook = lambda: _ntff_profile_via_ctypes(SO_PATH)
    sys.modules["antenv.axon_hooks"] = mod


def build_kernel(Tc, n_cores=8):
    import concourse.bass as bass  # noqa: F401
    import concourse.mybir as mybir
    import concourse.tile as tile
    from concourse import bacc
    from concourse.bass import ds, ts
    from concourse.masks import make_identity

    f32 = mybir.dt.float32
    bf16 = mybir.dt.bfloat16
    NG = Tc // TG  # groups of 512 tokens
    assert Tc % TG == 0

    nc = bacc.Bacc("TRN2", target_bir_lowering=False, debug=False, num_devices=n_cores)

    # host-pretransposed bf16 x: xt[gi, p, kb, t] = x[gi*TG + t, kb*128 + p]
    xt_in = nc.declare_dram_parameter("xt", [NG, 128, 8, TG], bf16, isOutput=False)
    wt_in = nc.declare_dram_parameter("wt", [128, 8, D], bf16, isOutput=False)
    g_in = nc.declare_dram_parameter("g", [128, 8, GCOLS], bf16, isOutput=False)
    u_in = nc.declare_dram_parameter("u", [ER + 1, D], bf16, isOutput=False)
    gb_in = nc.declare_dram_parameter("gb", [E, 1], f32, isOutput=False)
    # token-partition-major bf16 output (host casts back to fp32):
    # out_dev[gi, p, tt*D + o] = out[gi*TG + tt*128 + p, o]
    out_dram = nc.declare_dram_parameter(
        "out", [NG, 128, TT_PER_GROUP * D], bf16, isOutput=True)

    with tile.TileContext(nc) as tc:
        with contextlib.ExitStack() as ctx:
            singles = ctx.enter_context(tc.tile_pool(name="singles", bufs=1))
            smallp = ctx.enter_context(tc.tile_pool(name="smallp", bufs=2))
            h1p = ctx.enter_context(tc.tile_pool(name="h1p", bufs=2))
            outp = ctx.enter_context(tc.tile_pool(name="outp", bufs=3))
            po = ctx.enter_context(tc.tile_pool(name="po", bufs=5, space="PSUM"))
            pgp = ctx.enter_context(tc.tile_pool(name="pgp", bufs=1, space="PSUM"))
            pslp = ctx.enter_context(tc.tile_pool(name="pslp", bufs=1, space="PSUM"))
            pswp = ctx.enter_context(tc.tile_pool(name="pswp", bufs=1, space="PSUM"))

            # ---- constants / inputs ----
            g_sb = singles.tile([128, 8, GCOLS], bf16)
            u_sb = singles.tile([ER + 1, D], bf16)
            gb_sb = singles.tile([E, 1], f32)
            ident = singles.tile([128, 128], f32)
            ones = singles.tile([ER + 1, 1], f32)
            make_identity(nc, ident)
            nc.vector.memset(ones[:], 1.0)

            # sync ring: x group 0, then W (needed in token-tile order);
            # gpsimd SWDGE ring: x groups 1..3 (needed later)
            xt_sb = []
            for gi in range(NG):
                x_t = singles.tile([128, 8, TG], bf16, tag=f"xt{gi}")
                xt_sb.append(x_t)
            # ALL traffic on the sync ring, FIFO = priority order (the scalar
            # ring drip-feeds whenever the sync ring is loaded; a 40-byte gb
            # DMA once completed at 29us there and stalled the gating chain)
            nc.sync.dma_start(out=xt_sb[0][:], in_=xt_in[0])
            nc.sync.dma_start(out=g_sb[:], in_=g_in[:])
            nc.sync.dma_start(out=u_sb[:], in_=u_in[:])
            nc.sync.dma_start(out=gb_sb[:], in_=gb_in[:])
            NWC = 4  # W chunks (2 k-blocks each, 0.5 MB)
            wt_sb = []
            for c in range(NWC):
                w_t = singles.tile([128, 8 // NWC, D], bf16, tag=f"wt{c}")
                wt_sb.append(w_t)
                nc.sync.dma_start(out=w_t[:], in_=wt_in[:, ds(c * (8 // NWC), 8 // NWC), :])
            # later x groups follow W on the same FIFO ring: W is the
            # critical input, groups 1-3 aren't needed until 30/48/66us
            for gi in range(1, NG):
                nc.sync.dma_start(out=xt_sb[gi][:], in_=xt_in[gi])

            def wt_slice(kb, ch):
                c, kbi = divmod(kb, 8 // NWC)
                return wt_sb[c][:, kbi, ds(ch * 512, 512)]

            # ---- HAM warmup: fp32 identity matmuls (~500ns each) bridge the
            # PE from t~3us until the first x group lands (~11us), flipping
            # the HAM clock gate to 2.4GHz before real work starts ----
            warm_ps = pslp.tile([128, 128], f32, tag="psl")
            for _ in range(N_WARM):
                nc.tensor.matmul(warm_ps[:], ident, ident, start=True, stop=True)

            # ---- per-group phase A+B, software-pipelined into phase C ----
            def dg_matmul(gi):
                """down+gate merged matmul: pg rows 0:40 = h, 64:74/96:106 = logits hi/lo."""
                pg = pgp.tile([GCOLS, TG], f32, tag="pg")
                for kb in range(8):
                    nc.tensor.matmul(
                        pg[:], g_sb[:, kb, :], xt_sb[gi][:, kb, :],
                        start=(kb == 0), stop=(kb == 7),
                    )
                return pg

            def gate_front(gi, pg):
                """logits -> token-major psl transposes (PE) + hr evac."""
                hr = smallp.tile([ER, TG], f32, tag="hr")
                nc.scalar.copy(hr[:], pg[0:ER, :])
                # logits_hi + gate bias during PSUM evacuation (ACT), then +lo
                lt = smallp.tile([E, TG], f32, tag="lt")
                nc.scalar.activation(
                    lt[:], pg[GHI0:GHI0 + E, :],
                    mybir.ActivationFunctionType.Identity, bias=gb_sb[:],
                )
                lt2 = smallp.tile([E, TG], f32, tag="lt2")
                nc.vector.tensor_tensor(
                    lt2[:], lt[:], pg[GLO0:GLO0 + E, :], mybir.AluOpType.add,
                )
                psl = pslp.tile([128, TT_PER_GROUP, E], f32, tag="psl")
                for tt in range(TT_PER_GROUP):
                    nc.tensor.transpose(
                        psl[:, tt, :], lt2[:, ts(tt, 128)], ident[0:E, 0:E],
                    )
                return hr, psl

            def gate_back(gi, hr, psl):
                """top-2 softmax on tokens, rank-expand, transpose back, weight h."""
                L = smallp.tile([128, TT_PER_GROUP, E], f32, tag="L")
                nc.vector.tensor_copy(L[:], psl[:])
                m1 = smallp.tile([128, TT_PER_GROUP], f32, tag="m1")
                nc.vector.reduce_max(m1[:], L[:], axis=mybir.AxisListType.X)
                Lm = smallp.tile([128, TT_PER_GROUP, E], f32, tag="Lm")
                nc.vector.tensor_tensor(
                    Lm[:], L[:], m1[:, :, None].to_broadcast(L.shape),
                    mybir.AluOpType.subtract,
                )
                mmax = smallp.tile([128, TT_PER_GROUP, E], f32, tag="mmax")
                nc.vector.tensor_scalar(
                    mmax[:], Lm[:], 0.0, None, op0=mybir.AluOpType.is_equal,
                )
                nc.vector.tensor_scalar_mul(mmax[:], mmax[:], -1e30)
                nc.vector.tensor_tensor(mmax[:], Lm[:], mmax[:], mybir.AluOpType.add)
                m2 = smallp.tile([128, TT_PER_GROUP], f32, tag="m2")
                nc.vector.reduce_max(m2[:], mmax[:], axis=mybir.AxisListType.X)
                mask2 = smallp.tile([128, TT_PER_GROUP, E], f32, tag="mask2")
                nc.vector.tensor_tensor(
                    mask2[:], Lm[:], m2[:, :, None].to_broadcast(Lm.shape),
                    mybir.AluOpType.is_ge,
                )
                ex = smallp.tile([128, TT_PER_GROUP, E], f32, tag="ex")
                nc.scalar.activation(ex[:], Lm[:], mybir.ActivationFunctionType.Exp)
                nc.vector.tensor_tensor(ex[:], ex[:], mask2[:], mybir.AluOpType.mult)
                zsum = smallp.tile([128, TT_PER_GROUP], f32, tag="zsum")
                nc.vector.reduce_sum(zsum[:], ex[:], axis=mybir.AxisListType.X)
                nc.vector.reciprocal(zsum[:], zsum[:])
                wfull = smallp.tile([128, TT_PER_GROUP, E], f32, tag="wfull")
                nc.vector.tensor_tensor(
                    wfull[:], ex[:], zsum[:, :, None].to_broadcast(ex.shape),
                    mybir.AluOpType.mult,
                )
                w40 = smallp.tile([128, TT_PER_GROUP, ER], f32, tag="w40")
                nc.vector.tensor_copy(
                    w40[:],
                    wfull[:, :, :, None].to_broadcast([128, TT_PER_GROUP, E, R]),
                )
                psw = pswp.tile([ER, TG], f32, tag="psw")
                for tt in range(TT_PER_GROUP):
                    nc.tensor.transpose(
                        psw[:, ts(tt, 128)], w40[:, tt, :], ident,
                    )
                # h1 rows 0:40 = hr * w (bf16); row 40 = 1.0 (bias row of u)
                h1 = h1p.tile([ER + 1, TG], bf16, tag="h1")
                nc.vector.tensor_copy(h1[:], ones.to_broadcast([ER + 1, TG]))
                nc.vector.tensor_tensor(
                    h1[0:ER, :], hr[:], psw[:], mybir.AluOpType.mult,
                )
                return h1

            def token_tile(gi, tt, h1, o_sb, tj):
                """one 128-token tile: base matmuls + LoRA-up; evac split
                across ACT and DVE so neither queue's stalls gate PSUM
                bank recycling."""
                pout0 = po.tile([128, 512], f32, tag="po")
                pout1 = po.tile([128, 512], f32, tag="po")
                pouts = [pout0, pout1]
                for kb in range(8):
                    for ch in range(2):
                        nc.tensor.matmul(
                            pouts[ch][:], xt_sb[gi][:, kb, ts(tt, 128)],
                            wt_slice(kb, ch),
                            start=(kb == 0), stop=False,
                        )
                for ch in range(2):
                    nc.tensor.matmul(
                        pouts[ch][:], h1[:, ts(tt, 128)], u_sb[:, ds(ch * 512, 512)],
                        start=False, stop=True,
                    )
                nc.scalar.copy(o_sb[:, tj, ds(0, 512)], pouts[0][:])
                nc.vector.tensor_copy(o_sb[:, tj, ds(512, 512)], pouts[1][:])

            def pair_store(gi, ttp, o_sb):
                if gi == NG - 1 and ttp == 1:
                    # final store is the kernel tail: split partition halves
                    # across the sync + gpsimd rings so they transfer in
                    # parallel (~halves the tail latency)
                    nc.sync.dma_start(
                        out=out_dram[gi, 0:64, ds(ttp * 2 * D, 2 * D)],
                        in_=o_sb[0:64, :, :],
                    )
                    nc.gpsimd.dma_start(
                        out=out_dram[gi, 64:128, ds(ttp * 2 * D, 2 * D)],
                        in_=o_sb[64:128, :, :],
                    )
                else:
                    nc.sync.dma_start(
                        out=out_dram[gi, :, ds(ttp * 2 * D, 2 * D)], in_=o_sb[:],
                    )

            # prologue: group 0 gating (latency hides under the W-chunk loads)
            pg = dg_matmul(0)
            hr, psl = gate_front(0, pg)
            h1 = gate_back(0, hr, psl)
            for gi in range(NG):
                o_sb = outp.tile([128, 2, D], bf16, tag="o_sb")
                token_tile(gi, 0, h1, o_sb, 0)
                token_tile(gi, 1, h1, o_sb, 1)
                pair_store(gi, 0, o_sb)
                if gi + 1 < NG:
                    pg_n = dg_matmul(gi + 1)
                    hr_n, psl_n = gate_front(gi + 1, pg_n)
                    h1_n = gate_back(gi + 1, hr_n, psl_n)
                o_sb = outp.tile([128, 2, D], bf16, tag="o_sb")
                token_tile(gi, 2, h1, o_sb, 0)
                token_tile(gi, 3, h1, o_sb, 1)
                pair_store(gi, 1, o_sb)
                if gi + 1 < NG:
                    h1 = h1_n

    nc.compile()
    return nc


def pack_weights(W_base, b_base, gate_W, gate_b, lora_down, lora_up):
    """Host-side packing of the replicated weights into device layouts (bf16)."""
    import ml_dtypes
    bf = ml_dtypes.bfloat16
    W_base = np.asarray(W_base, np.float32)
    b_base = np.asarray(b_base, np.float32)
    gate_W = np.asarray(gate_W, np.float32)
    gate_b = np.asarray(gate_b, np.float32)
    lora_down = np.asarray(lora_down, np.float32)
    lora_up = np.asarray(lora_up, np.float32)

    # wt[p, kb, o] = W^T[d, o] = W_base[o, d], d = kb*128+p
    wt = np.ascontiguousarray(
        np.ascontiguousarray(W_base.T).reshape(8, 128, D).transpose(1, 0, 2)
    ).astype(bf)
    # merged lhsT: cols 0..39 lora_down^T, 64..73 gate_hi, 96..105 gate_lo.
    # gate_W is split hi/lo in bf16 so the two column sets sum to ~16-bit
    # gate precision (x itself is bf16).
    gwT = np.ascontiguousarray(gate_W.T)                  # [D, E]
    gw_hi = gwT.astype(bf).astype(np.float32)
    gw_lo = (gwT - gw_hi).astype(bf).astype(np.float32)
    G = np.zeros((D, GCOLS), np.float32)
    G[:, 0:ER] = lora_down.reshape(ER, D).T
    G[:, GHI0:GHI0 + E] = gw_hi
    G[:, GLO0:GLO0 + E] = gw_lo
    g = np.ascontiguousarray(G.reshape(8, 128, GCOLS).transpose(1, 0, 2)).astype(bf)
    # u rows 0..39: lora_up[e, o, r]*0.25 -> [er, o]; row 40: b_base
    U = lora_up.transpose(0, 2, 1).reshape(ER, D) * (1.0 / R)
    u = np.ascontiguousarray(np.concatenate([U, b_base[None, :]], axis=0)).astype(bf)
    gb = np.ascontiguousarray(gate_b[:, None])
    return {"wt": wt, "g": g, "u": u, "gb": gb}


def pack_x(xc, Tc):
    """Per-core x [Tc, D] -> bf16 [NG, 128, 8, TG]: xt[gi,p,kb,t]=x[gi*TG+t, kb*128+p]."""
    import ml_dtypes
    NG = Tc // TG
    return np.ascontiguousarray(
        xc.reshape(NG, TG, 8, 128).transpose(0, 3, 2, 1)
    ).astype(ml_dtypes.bfloat16)


def unpack_out(o_dev, Tc):
    """Device layout bf16 [NG, 128, TT*D] -> fp32 [Tc, D]."""
    NG = Tc // TG
    return np.ascontiguousarray(
        o_dev.reshape(NG, 128, TT_PER_GROUP, D).transpose(0, 2, 1, 3)
    ).reshape(Tc, D).astype(np.float32)


def run(nc, inputs, Tc, n_cores=8, trace=False):
    """Shard x over cores, run SPMD, gather output."""
    from concourse.bass_utils import run_bass_kernel_spmd

    x = np.asarray(inputs["x"], np.float32)
    B, S, _ = x.shape
    xf = x.reshape(B * S, D)
    assert B * S == Tc * n_cores
    packed = pack_weights(
        inputs["W_base"], inputs["b_base"], inputs["gate_W"],
        inputs["gate_b"], inputs["lora_down"], inputs["lora_up"],
    )
    in_maps = [
        {"xt": pack_x(xf[c * Tc:(c + 1) * Tc], Tc), **packed}
        for c in range(n_cores)
    ]
    kwargs = {}
    if trace:
        install_ntff_hook()
        kwargs = {"trace": True}
    res = run_bass_kernel_spmd(nc, in_maps, core_ids=list(range(n_cores)), **kwargs)
    out = np.concatenate(
        [unpack_out(res.results[c]["out"], Tc) for c in range(n_cores)], axis=0)
    return out.reshape(B, S, D), res


_NC_CACHE = {}


def kernel(**inputs):
    """Full-input MoE-LoRA forward on 8 TRN2 NeuronCores (token-parallel).

    Takes the unsharded inputs from setup_inputs(), returns [B, S, D] fp32.
    """
    x = np.asarray(inputs["x"], np.float32)
    B, S, _ = x.shape
    n_cores = 8
    total = B * S
    assert total % n_cores == 0
    Tc = total // n_cores
    key = (Tc, n_cores)
    if key not in _NC_CACHE:
        _NC_CACHE[key] = build_kernel(Tc, n_cores=n_cores)
    nc = _NC_CACHE[key]
    last_err = None
    for _ in range(3):  # transient device wedges recover on retry
        try:
            out, _res = run(nc, inputs, Tc, n_cores=n_cores)
            return out
        except Exception as e:  # noqa: BLE001
            last_err = e
            import time as _time
            _time.sleep(5)
    raise last_err
